# revision 8
# baseline (speedup 1.0000x reference)
"""Trainium2 Bass kernel: batched multi-head scaled-dot-product attention.

Problem shapes: Q/K/V [4, 16, 2048, 64] f32, mask [4, 1, 2048, 2048] bool.
out = softmax(Q K^T / 8 + mask) V.

Sharding: 8 cores; core c handles batch b = c//2, heads (c%2)*8 .. +8.
Each (b, h) is independent; the mask is shared across the 8 heads on a core.

Per-core kernel (per head h, per query-half qh of 1024):
  S^T[k, q]  = K Q^T        (PE; d=64 on partitions, 16 key-blocks of 128)
  P^T        = exp(S^T / 8)  (ACT, from PSUM)
  P^T       *= mask^T (0/1)  (DVE + GPSIMD split; bf16 mask, exact)
  acc[d', q] = sum_k V'[k, d'] P^T[k, q]   (PE, PSUM accumulate over k-blocks;
               V' has a ones column so acc[64, q] = softmax denominator)
  out[q, d]  = transpose(acc) * 1/acc[64]  (PE transpose + DVE recip/mul)

Matmuls run as float32r (bitcast) for 4x PE throughput vs float32.
"""

import numpy as np
import ml_dtypes

import bass_rust
import concourse.bass as bass
import concourse.mybir as mybir
import concourse.tile as tile
from concourse.bass_utils import run_bass_kernel_spmd
from concourse.masks import make_identity

B, H, S, D = 4, 16, 2048, 64
N_CORES = 8
HPC = H // (N_CORES // B)  # heads per core = 8
KB = S // 128  # 16 key blocks
QH = 2  # query halves
QHS = S // QH  # 1024
F32 = mybir.dt.float32
BF16 = mybir.dt.bfloat16

# float32r: 1 cycle/row on PE (vs 4 for float32) when the moving dim >= 256.
MM_DT = mybir.dt.float32r
# Columns of each 1024-wide mask multiply handled by DVE; rest on GPSIMD.
DVE_COLS = 512


def _patched_drain_and_barrier(self, tick_clock, wait_clock):
    """This neuronxcc's CoreV3 codegen allows only 1 sync-wait per TPB_CTRL
    instruction; Tile's end-of-kernel drain can carry many. Split them."""
    drain_inst = self.nc.sync.drain()
    wait_clock.add_sem_waits(
        drain_inst.ins, tile.ScopedClock({None: tick_clock.global_clock})
    )
    mi = drain_inst.ins
    si = mi.sync_info
    waits = list(si.on_wait) if si is not None else []
    if len(waits) > 1:
        si.on_wait = waits[:1]
        mi.sync_info = si
        for i in range(1, len(waits)):
            extra = self.nc.sync.drain()
            extra.ins.sync_info = bass_rust.SyncInfo(
                on_wait=waits[i : i + 1], on_update=[]
            )
    self.nc.all_engine_barrier()
    popped = self.nc._tile_sem_poison_stack.pop()
    assert popped is self._sem_poison
    self.nc.clear_and_free_semaphores(list(self.sems.allocated().values()))
    self.nc.all_engine_barrier()


tile.TileContext._drain_and_barrier = _patched_drain_and_barrier

_ORIG_COMMIT = tile.TileContext._commit_instruction


def _commit_split_waits(self, inst, lazy_reg_writes=True):
    """Hoist all-but-one sem wait of a compute instruction onto single-wait
    NoOp carriers on the same engine (same 1-wait codegen limit as above).
    DMACopy here lowers to SP pseudo-DMA instructions, dispatched in program
    order by the SP sequencer, so carriers gate them too."""
    si = getattr(inst, "sync_info", None)
    if (
        si is not None
        and len(si.on_wait) > 1
        and inst.engine != mybir.EngineType.Unassigned
    ):
        waits = list(si.on_wait)
        for w in waits[:-1]:
            nop = mybir.InstNoOp(name=self.nc.get_next_instruction_name())
            nop.engine = inst.engine
            nop.sync_info = bass_rust.SyncInfo(on_wait=[w], on_update=[])
            self._add_instruction(nop)
        si.on_wait = waits[-1:]
        inst.sync_info = si
    return _ORIG_COMMIT(self, inst, lazy_reg_writes)


tile.TileContext._commit_instruction = _commit_split_waits

_NC_CACHE = {}


def build_nc():
    if "nc" in _NC_CACHE:
        return _NC_CACHE["nc"]
    nc = bass.Bass("TRN2", target_bir_lowering=False, debug=False, num_devices=N_CORES)
    qT = nc.dram_tensor("qT", [HPC, D, S], MM_DT, kind="ExternalInput")
    kT = nc.dram_tensor("kT", [HPC, D, S], MM_DT, kind="ExternalInput")
    v = nc.dram_tensor("v", [HPC, S, D + 1], MM_DT, kind="ExternalInput")
    maskT = nc.dram_tensor("maskT", [S, S], BF16, kind="ExternalInput")
    out = nc.dram_tensor("out", [HPC, S, D], F32, kind="ExternalOutput")

    with tile.TileContext(nc) as tc:
        with (
            tc.tile_pool(name="consts", bufs=1) as consts,
            tc.tile_pool(name="qk", bufs=2) as qk_pool,
            tc.tile_pool(name="vp", bufs=2) as v_pool,
            tc.tile_pool(name="pp", bufs=3) as p_pool,
            tc.tile_pool(name="op", bufs=2) as o_pool,
            tc.tile_pool(name="small", bufs=2) as small,
            tc.tile_pool(name="res", bufs=2) as res_pool,
            tc.tile_pool(name="ps_s", bufs=2, space="PSUM") as ps_s,
            tc.tile_pool(name="ps_acc", bufs=1, space="PSUM") as ps_acc,
            tc.tile_pool(name="ps_tr", bufs=1, space="PSUM") as ps_tr,
        ):
            identity = consts.tile([128, 128], F32)
            make_identity(nc, identity)
            mask_sb = consts.tile([128, KB, S], BF16)
            mT = maskT.rearrange("(n p) q -> p n q", p=128)
            for kb in range(KB):
                nc.sync.dma_start(out=mask_sb[:, kb, :], in_=mT[:, kb, :])

            for h in range(HPC):
                qT_sb = qk_pool.tile([D, S], MM_DT, tag="q", name=f"qT_{h}")
                kT_sb = qk_pool.tile([D, S], MM_DT, tag="k", name=f"kT_{h}")
                nc.sync.dma_start(out=qT_sb, in_=qT[h])
                nc.sync.dma_start(out=kT_sb, in_=kT[h])
                v_sb = v_pool.tile([128, KB, D + 1], MM_DT, tag="v", name=f"v_{h}")
                nc.sync.dma_start(
                    out=v_sb, in_=v[h].rearrange("(n p) d -> p n d", p=128)
                )

                for qh in range(QH):
                    q0 = qh * QHS
                    acc = ps_acc.tile([D + 1, QHS], F32, tag="acc", name=f"acc_{h}_{qh}")
                    for kb in range(KB):
                        s_t = ps_s.tile([128, QHS], F32, tag="s", name=f"s_{h}_{qh}_{kb}")
                        for j in range(2):
                            nc.tensor.matmul(
                                s_t[:, j * 512 : (j + 1) * 512],
                                kT_sb[:, kb * 128 : (kb + 1) * 128],
                                qT_sb[:, q0 + j * 512 : q0 + (j + 1) * 512],
                                start=True,
                                stop=True,
                            )
                        p_t = p_pool.tile([128, QHS], MM_DT, tag="p", name=f"p_{h}_{qh}_{kb}")
                        nc.scalar.activation(
                            p_t, s_t, mybir.ActivationFunctionType.Exp, scale=0.125
                        )
                        m_ap = mask_sb[:, kb, q0 : q0 + QHS]
                        nc.vector.tensor_mul(
                            p_t[:, 0:DVE_COLS], p_t[:, 0:DVE_COLS], m_ap[:, 0:DVE_COLS]
                        )
                        nc.gpsimd.tensor_mul(
                            p_t[:, DVE_COLS:], p_t[:, DVE_COLS:], m_ap[:, DVE_COLS:]
                        )
                        for j in range(2):
                            nc.tensor.matmul(
                                acc[:, j * 512 : (j + 1) * 512],
                                v_sb[:, kb, :],
                                p_t[:, j * 512 : (j + 1) * 512],
                                start=(kb == 0),
                                stop=(kb == KB - 1),
                            )
                    # normalize: transpose acc back to [q, d], divide by ones-col
                    o_sb = o_pool.tile([D + 1, QHS], F32, tag="o", name=f"o_{h}_{qh}")
                    nc.vector.tensor_copy(o_sb, acc)
                    tr = ps_tr.tile([128, 8, 128], F32, tag="tr", name=f"tr_{h}_{qh}")
                    for j in range(8):
                        nc.tensor.transpose(
                            tr[:, j, 0 : D + 1],
                            o_sb[:, j * 128 : (j + 1) * 128],
                            identity[0 : D + 1, 0 : D + 1],
                        )
                    den_sb = small.tile([128, 8], F32, tag="den", name=f"den_{h}_{qh}")
                    nc.vector.tensor_copy(den_sb, tr[:, :, D])
                    rec_sb = small.tile([128, 8], F32, tag="rec", name=f"rec_{h}_{qh}")
                    nc.vector.reciprocal(rec_sb, den_sb)
                    res_sb = res_pool.tile([128, 8, D], F32, tag="res", name=f"res_{h}_{qh}")
                    for j in range(8):
                        nc.vector.tensor_scalar_mul(
                            res_sb[:, j, :], tr[:, j, 0:D], rec_sb[:, j : j + 1]
                        )
                    nc.sync.dma_start(
                        out=out[h, q0 : q0 + QHS, :].rearrange(
                            "(j p) d -> p j d", p=128
                        ),
                        in_=res_sb,
                    )
    _NC_CACHE["nc"] = nc
    return nc


def make_in_maps(encodings_q, encodings_k, encodings_v, mask):
    in_maps = []
    maskT_by_b = {}
    for b in range(B):
        maskT_by_b[b] = np.ascontiguousarray(mask[b, 0].T).astype(ml_dtypes.bfloat16)
    for c in range(N_CORES):
        b = c // (N_CORES // B)
        h0 = (c % (N_CORES // B)) * HPC
        in_maps.append(
            {
                "qT": np.ascontiguousarray(
                    encodings_q[b, h0 : h0 + HPC].transpose(0, 2, 1)
                ),
                "kT": np.ascontiguousarray(
                    encodings_k[b, h0 : h0 + HPC].transpose(0, 2, 1)
                ),
                "v": np.concatenate(
                    [
                        encodings_v[b, h0 : h0 + HPC],
                        np.ones((HPC, S, 1), np.float32),
                    ],
                    axis=-1,
                ),
                "maskT": maskT_by_b[b],
            }
        )
    return in_maps


def gather_out(results):
    out = np.empty((B, H, S, D), np.float32)
    for c in range(N_CORES):
        b = c // (N_CORES // B)
        h0 = (c % (N_CORES // B)) * HPC
        out[b, h0 : h0 + HPC] = results[c]["out"]
    return out


def kernel(encodings_q, encodings_k, encodings_v, mask):
    nc = build_nc()
    in_maps = make_in_maps(encodings_q, encodings_k, encodings_v, mask)
    res = run_bass_kernel_spmd(nc, in_maps, core_ids=list(range(N_CORES)))
    return gather_out(res.results)
